# revision 7
# baseline (speedup 1.0000x reference)
"""Trainium2 Bass kernel for PhinRhythmAttention (B=2,S=2048,D=1024,H=16).

Sharding: tensor-parallel over heads. Core c owns heads {2c, 2c+1} (dh=128
combined) for both batches: qkv projection column slice, full attention for
those heads, and the out-projection row slice. Host sums the 8 partial
out-projection results and adds out_b.

On-chip layout is feature-major (transposed): xT [1024,4096] built via PE
transposes, qT/kT/vT [128,S] per batch, scoresT = kT_chunk.T @ qT, softmax
without max-subtraction (scores are small; +0.01 is shift-invariant and
dropped), denominator via a ones-column appended to v in the ctx matmul.
All PE matmuls run in float32r.
"""

import numpy as np
from contextlib import ExitStack

import concourse.bacc as bacc
import concourse.tile as tile
from concourse import mybir
from concourse.bass_utils import run_bass_kernel_spmd

B, S, D, H = 2, 2048, 1024, 16
NT = B * S
f32 = mybir.dt.float32
f32r = mybir.dt.float32r

_cache = {}


def build(repeat=1):
    nc = bacc.Bacc()
    x_d = nc.dram_tensor("x", [NT, D], f32r, kind="ExternalInput")
    wq_d = nc.dram_tensor("wq", [D, 128], f32r, kind="ExternalInput")
    wk_d = nc.dram_tensor("wk", [D, 128], f32r, kind="ExternalInput")
    wv_d = nc.dram_tensor("wv", [D, 128], f32r, kind="ExternalInput")
    bq_d = nc.dram_tensor("bq", [128, 1], f32, kind="ExternalInput")
    bk_d = nc.dram_tensor("bk", [128, 1], f32, kind="ExternalInput")
    bv_d = nc.dram_tensor("bv", [128, 1], f32, kind="ExternalInput")
    wo_d = nc.dram_tensor("wo", [128, D], f32r, kind="ExternalInput")
    id_d = nc.dram_tensor("ident", [128, 128], f32r, kind="ExternalInput")
    outT_d = nc.dram_tensor("outT", [D, NT], f32, kind="ExternalOutput")

    with tile.TileContext(nc) as tc, ExitStack() as ctx:
        sb = ctx.enter_context(tc.tile_pool(name="sb", bufs=1))
        xsp = ctx.enter_context(tc.tile_pool(name="xsp", bufs=2))
        etp = ctx.enter_context(tc.tile_pool(name="etp", bufs=3))
        dp = ctx.enter_context(tc.tile_pool(name="dp", bufs=2))
        otp = ctx.enter_context(tc.tile_pool(name="otp", bufs=2))
        tps = ctx.enter_context(tc.tile_pool(name="tps", space="PSUM", bufs=3))
        sps = ctx.enter_context(tc.tile_pool(name="sps", space="PSUM", bufs=2))
        cps = ctx.enter_context(tc.tile_pool(name="cps", space="PSUM", bufs=1))
        pps = ctx.enter_context(tc.tile_pool(name="pps", space="PSUM", bufs=2))

        ident = sb.tile([128, 128], f32r)
        nc.sync.dma_start(out=ident, in_=id_d[:, :])
        ones = sb.tile([128, 2], f32)
        nc.vector.memset(ones, 1.0)

        w_sb, b_sb = {}, {}
        for nm, wd, bd in (("q", wq_d, bq_d), ("k", wk_d, bk_d), ("v", wv_d, bv_d)):
            wt = sb.tile([128, 8, 128], f32r, name=f"w{nm}")
            for kk in range(8):
                nc.sync.dma_start(out=wt[:, kk, :], in_=wd[kk * 128:(kk + 1) * 128, :])
            w_sb[nm] = wt
            bt = sb.tile([128, 1], f32, name=f"b{nm}")
            nc.sync.dma_start(out=bt, in_=bd[:, :])
            b_sb[nm] = bt
        wo_sb = sb.tile([128, D], f32r)
        nc.sync.dma_start(out=wo_sb, in_=wo_d[:, :])

        xT = [sb.tile([128, S], f32r, name=f"xT{i}") for i in range(8)]
        qkvT = {nm: sb.tile([128, S], f32r, name=f"{nm}T") for nm in ("q", "k", "v")}
        vs = [[sb.tile([128, 66], f32r, name=f"vs{h}_{i}") for i in range(16)]
              for h in range(2)]
        ctxn = sb.tile([128, S], f32r)

        for _rep in range(repeat):
            for b in range(B):
                # Phase A: xT = x[b].T (feature-major), via PE transposes
                for si in range(16):
                    xs = xsp.tile([128, D], f32r, tag="xs")
                    r0 = b * S + si * 128
                    nc.sync.dma_start(out=xs, in_=x_d[r0:r0 + 128, :])
                    for fi in range(8):
                        pt = tps.tile([128, 128], f32r, tag="pt")
                        nc.tensor.transpose(pt, xs[:, fi * 128:(fi + 1) * 128], ident)
                        nc.vector.tensor_copy(
                            out=xT[fi][:, si * 128:(si + 1) * 128], in_=pt)

                # Phase B: qT/kT/vT = W.T @ xT + b (q pre-scaled by 1/8)
                for nm in ("q", "k", "v"):
                    for sj in range(4):
                        pp = pps.tile([128, 512], f32, tag="pp")
                        for kk in range(8):
                            nc.tensor.matmul(
                                pp, w_sb[nm][:, kk, :],
                                xT[kk][:, sj * 512:(sj + 1) * 512],
                                start=(kk == 0), stop=(kk == 7))
                        nc.vector.tensor_scalar_add(
                            out=qkvT[nm][:, sj * 512:(sj + 1) * 512],
                            in0=pp, scalar1=b_sb[nm])

                # Phase C: v back to token-major per head, with ones column
                for si in range(16):
                    pv = tps.tile([128, 128], f32r, tag="pt")
                    nc.tensor.transpose(
                        pv, qkvT["v"][:, si * 128:(si + 1) * 128], ident)
                    for h in range(2):
                        nc.vector.tensor_copy(
                            out=vs[h][si][:, 0:64], in_=pv[:, h * 64:h * 64 + 64])
                        nc.vector.tensor_copy(
                            out=vs[h][si][:, 64:65], in_=ones[:, 0:1])

                # Phase D: attention per head
                for h in range(2):
                    qh = qkvT["q"][h * 64:(h + 1) * 64, :]
                    kh = qkvT["k"][h * 64:(h + 1) * 64, :]
                    for sq in range(4):
                        ctxp = cps.tile([65, 512], f32, tag="c")
                        for sk in range(16):
                            sp = sps.tile([128, 512], f32, tag="s")
                            nc.tensor.matmul(
                                sp, kh[:, sk * 128:(sk + 1) * 128],
                                qh[:, sq * 512:(sq + 1) * 512],
                                start=True, stop=True)
                            et = etp.tile([128, 512], f32r, tag="e")
                            nc.scalar.activation(
                                out=et, in_=sp,
                                func=mybir.ActivationFunctionType.Exp)
                            nc.tensor.matmul(
                                ctxp, vs[h][sk][:, 0:65], et,
                                start=(sk == 0), stop=(sk == 15))
                        # normalize: rows 0..63 ctxT, row 64 denominator
                        cd = dp.tile([65, 512], f32r, tag="cd")
                        nc.vector.tensor_copy(out=cd, in_=ctxp)
                        for t in range(4):
                            tp = tps.tile([128, 128], f32r, tag="pt")
                            nc.tensor.transpose(
                                tp[:, 0:66], cd[:, t * 128:(t + 1) * 128],
                                ident[0:65, 0:66])
                            cb = dp.tile([128, 65], f32, tag="cb")
                            nc.vector.tensor_copy(out=cb, in_=tp[:, 0:65])
                            rc = dp.tile([128, 1], f32, tag="rc")
                            nc.vector.reciprocal(out=rc, in_=cb[:, 64:65])
                            cn = dp.tile([128, 64], f32r, tag="cn")
                            nc.vector.tensor_scalar_mul(
                                out=cn, in0=cb[:, 0:64], scalar1=rc)
                            tp2 = tps.tile([128, 128], f32r, tag="pt")
                            nc.tensor.transpose(tp2[0:64, 0:128], cn, ident)
                            nc.vector.tensor_copy(
                                out=ctxn[h * 64:(h + 1) * 64,
                                         sq * 512 + t * 128:sq * 512 + (t + 1) * 128],
                                in_=tp2[0:64, 0:128])

                # Phase E: partial out-projection outT = wo.T @ ctxn
                for sj in range(4):
                    for e in range(8):
                        op = pps.tile([128, 512], f32, tag="pp")
                        nc.tensor.matmul(
                            op, wo_sb[:, e * 128:(e + 1) * 128],
                            ctxn[:, sj * 512:(sj + 1) * 512],
                            start=True, stop=True)
                        ot = otp.tile([128, 512], f32, tag="ot")
                        nc.vector.tensor_copy(out=ot, in_=op)
                        nc.sync.dma_start(
                            out=outT_d[e * 128:(e + 1) * 128,
                                       b * S + sj * 512:b * S + (sj + 1) * 512],
                            in_=ot)
    nc.finalize()
    return nc


def make_in_maps(x, qkv_w, qkv_b, out_w):
    x2 = np.ascontiguousarray(np.asarray(x, dtype=np.float32).reshape(NT, D))
    qkv_w = np.asarray(qkv_w, dtype=np.float32)
    qkv_b = np.asarray(qkv_b, dtype=np.float32)
    out_w = np.asarray(out_w, dtype=np.float32)
    ident = np.eye(128, dtype=np.float32)
    in_maps = []
    for c in range(8):
        a = 128 * c
        in_maps.append({
            "x": x2,
            "wq": np.ascontiguousarray(qkv_w[:, a:a + 128]) * np.float32(0.125),
            "wk": np.ascontiguousarray(qkv_w[:, D + a:D + a + 128]),
            "wv": np.ascontiguousarray(qkv_w[:, 2 * D + a:2 * D + a + 128]),
            "bq": (qkv_b[a:a + 128] * np.float32(0.125)).reshape(128, 1).copy(),
            "bk": qkv_b[D + a:D + a + 128].reshape(128, 1).copy(),
            "bv": qkv_b[2 * D + a:2 * D + a + 128].reshape(128, 1).copy(),
            "wo": np.ascontiguousarray(out_w[a:a + 128, :]),
            "ident": ident,
        })
    return in_maps


def kernel(x, rhythm_pattern, qkv_w, qkv_b, out_w, out_b, rhythm_weights):
    if "nc" not in _cache:
        _cache["nc"] = build()
    nc = _cache["nc"]
    in_maps = make_in_maps(x, qkv_w, qkv_b, out_w)
    res = run_bass_kernel_spmd(nc, in_maps, list(range(8)), trace=False)
    outT = res.results[0]["outT"].copy()
    for c in range(1, 8):
        outT += res.results[c]["outT"]
    out = outT.T + np.asarray(out_b, dtype=np.float32)
    return np.ascontiguousarray(out.reshape(B, S, D).astype(np.float32))
